# revision 46
# baseline (speedup 1.0000x reference)
"""Trainium2 Bass kernel for nn_DigitConvolutionalModel (dense_cnn).

Model: y = relu(conv3x3(x) @ w1.T + b1) @ w2.T + b2, x: [65536, 784] f32.

Strategy:
  * The 3x3 valid conv (784 -> 676) and FC1 (676 -> 128) are both linear,
    so they fuse on the host into one effective weight W1e = w1 @ C with
    shape [128, 784] (C is the sparse conv operator). The device then runs
    a pure GEMM pipeline: y = relu(x @ W1e.T + b1) @ w2.T + b2.
  * Pure data parallel over 8 NeuronCores: each core gets 8192 rows of x.
    No collectives; each core produces its own output shard.
  * Mixed-precision operands (all accumulation fp32 in PSUM): features
    0:512 travel fp16, features 512:768 travel fp8e4m3 against fp16
    weights (mixed-dtype matmul, HW-verified bit-exact vs the quantized
    reference), the 16-feature tail travels fp16. End-to-end rel_fro =
    1.402e-2 on the fixed inputs (deterministic; HW matches the numpy
    emulation to 4 digits), under the 2e-2 gate with 30% margin, and x
    HBM bytes drop from 12.6 to 10.6 MB/core. More fp8 (n8=400) would
    leave only 10% margin -- not taken.
  * Every x DMA uses exactly 128 partitions -> one contiguous
    descriptor per partition, port-matched by the HWDGE balancer to the
    16 SDMA engines (~25 GB/s/engine). Measured: any other partition
    count (16- or 112-partition tiles) breaks the port match and drops
    to ~16 GB/s/engine. fp16 loads taper 512,512,1024x6,512,512 cols
    (quick first block, short drain; 1024-col mid loads keep PE idle
    gaps under HAM's 3.4us re-throttle window -- 2048-col loads caused
    mid-kernel 1.2 GHz collapses). fp8 chunks ride 4 x [128, 2, 2048]
    loads interleaved into the stream. All loads are SBUF-resident (no
    recycling -> the stream never waits on compute). Sync-ring order
    [w1p, x0, x8_0, wrest, x1, ...] keeps total HWDGE DMAs at 16 so the
    8 round-robin DMAHW semaphore lanes only pair early-completing
    loads (lane reuse stalled loads 10-30us in earlier revisions).
  * The 16-feature tail for the whole batch travels inside wrest as a
    [128, 2048] block: block bi's tail sits on the 32-aligned partition
    strip 32*tail_strip(bi). A pair's two K=16 tail matmuls land in
    different row groups and execute concurrently via tile_position row
    tiling (~4ns apart, measured).
  * Blocks process in weight-stationary pairs (one 1024-col load): 12
    main matmuls stream at the PE's pure 216ns cadence, 2 concurrent
    tail matmuls, relu+bias (PSUM -> SBUF fp16) alternating DVE/ACT so
    the pair's two relus run in parallel, then the pair's two [10, 512]
    FC2 matmuls pack into one PSUM bank at col strips (0,0)/(0,32) and
    run concurrently. FC2 bias rides a DVE tensor_scalar (80ns vs 677ns
    on ACT). FC2 groups are emitted one group behind FC1 so relu
    latency hides under the next group's matmuls. Output returns as
    yT [10, 8192] per core; the host transposes.
  * Stores issue via SWDGE from the otherwise-idle gpsimd engine: its
    DMASW semaphore lanes are separate from the x stream's DMAHW lanes,
    and an ACT-issued HWDGE store costs 1.4us of sequencer time.
  * The PE HAM clock gate defaults to 1.2 GHz and needs ~3.4us of
    sustained activity to ramp to 2.4 GHz. Twelve dummy matmuls bridge
    the startup window (event-semaphore preamble + first loads), and
    always-ready filler matmuls after the first four groups keep the PE
    busy through early x-load waits -- without them HAM re-throttles
    mid-kernel for ~7us (measured).
  * Tile/walrus quirks handled explicitly: this walrus allows ONE sync
    wait per instruction, so multi-waits are split into event-semaphore
    chains (bass_rust.generate_event_semaphores) and tiny dummy
    ldweights "probes" absorb cross-engine waits into the PE stream
    ahead of each matmul group.
"""

import numpy as np

import concourse.bass as bass
import concourse.mybir as mybir
import concourse.tile as tile
from concourse.bass import ts
from concourse.bass_utils import run_bass_kernel_spmd

H = W = 28
KH = KW = 3
CIN = H * W  # 784
HID = 128
OUT = 10
B_TOTAL = 65536
NCORES = 8
BS = B_TOTAL // NCORES  # 8192 rows per core
NB = 512  # batch columns per psum block (fp32 PSUM bank limit)
NBLK = BS // NB  # 16
KCH = 128
KC = 6  # full chunks (6 * 128 = 768)
KC16 = 4  # chunks 0..3 (features 0:512) travel fp16
KC8 = KC - KC16  # chunks 4,5 (features 512:768) travel fp8e4m3
KTAIL = CIN - KC * KCH  # 16
NGRP = 4  # tail partition strips, at 32-aligned bases {0,32,64,96}
NWARM = 12  # HAM warm-up dummy matmuls
X8COLS = 2048  # columns per fp8 x load (4 loads cover the batch)


def tail_strip(bi):
    """Tail strip index for block bi: a pair [2k, 2k+1] lands on strips
    in different 32-row groups, so the two K=16 tail matmuls run
    concurrently via tile_position row tiling."""
    return 2 * (bi % 2) + (bi // 2) % 2


def tail_colrange(bi):
    return bi // NGRP

# x load schedule (column start, width). Fine-grained 1024-col loads in
# the middle: the PE consumes a 1024-col pair in ~3.9us while the DMA
# delivers one in ~3.7us, so the PE never idles past HAM's 3.4us MID
# window (a 2048-col load creates a >3.5us PE bubble and the clock gate
# drops to 1.2 GHz -- measured in v6). 512-col loads at both ends for a
# fast first block and a short drain. 12 HWDGE DMAs (w1p, x0, wrest,
# x1..x9) over 8 round-robin DMAHW lanes: the four lane reuses all pair
# a late x load with an early-completing load, so no issue stalls.
LOADS = (
    [(0, 512), (512, 512)]
    + [(cs, 1024) for cs in range(1024, BS - 1024, 1024)]
    + [(BS - 1024, 512), (BS - 512, 512)]
)
assert sum(n for _, n in LOADS) == BS

# weight DMAs split so the sync-ring order [w1p, x0, wrest, x1, ...]
# gates block 0's main FC1 only on the tiny w1p + x0 loads:
#   w1p  [128, 768]:  w1e main chunks [k, c, m] (c-major)
#   wrest [128, 2186]: cols 0:128 = tail weight rows replicated on all 4
#                      32-aligned strips (the tail matmuls slice 16-row
#                      windows, so one copy serves every strip),
#                      128:138 = w2t, 138:2186 = per-core packed x tail
W1COLS = 768
TAILOFF, W2OFF, XTLOFF, WRCOLS = 0, 128, 138, 2186

HOST_DT = np.float16


def _build_nc():
    f32 = mybir.dt.float32
    mdt = mybir.dt.float16
    f8 = mybir.dt.float8e4
    nc = bass.Bass()
    xts = [
        nc.dram_tensor(f"x{li}", [KCH, KC16, ncols], mdt, kind="ExternalInput")
        for li, (_, ncols) in enumerate(LOADS)
    ]
    x8ts = [
        nc.dram_tensor(f"x8_{k}", [KCH, KC8, X8COLS], f8, kind="ExternalInput")
        for k in range(BS // X8COLS)
    ]
    w1p = nc.dram_tensor("w1p", [KCH, W1COLS], mdt, kind="ExternalInput")
    wrest = nc.dram_tensor("wrest", [KCH, WRCOLS], mdt, kind="ExternalInput")
    # both biases in one f32 tensor: col 0 = b1, col 1 rows 0:10 = b2
    bd = nc.dram_tensor("bd", [HID, 2], f32, kind="ExternalInput")
    yt = nc.dram_tensor("yt", [OUT, BS], f32, kind="ExternalOutput")

    with tile.TileContext(nc) as tc:
        with (
            tc.tile_pool(name="consts", bufs=1) as consts,
            tc.tile_pool(name="xin", bufs=1) as xin,
            tc.tile_pool(name="hpool", bufs=8) as hpool,
            tc.tile_pool(name="opool", bufs=6) as opool,
            tc.tile_pool(name="ps1", bufs=4, space="PSUM") as ps1p,
            tc.tile_pool(name="ps2", bufs=2, space="PSUM") as ps2p,
            tc.tile_pool(name="psd", bufs=1, space="PSUM") as psdp,
        ):
            # Sync-ring order [w1p, x0, wrest, x1, ...]: the 0.23 MB main
            # weights land first (~10us), x0 right behind (~12us), and the
            # tail/w2/xtl pack arrives just before block 0's tail matmul
            # needs it. (v4's weights-on-scalar crawled behind the x
            # stream to ~18us; v5's full-pack-first pushed x0 late and let
            # HAM re-throttle during the 4us PE idle that followed.)
            w1p_t = consts.tile([KCH, W1COLS], mdt)
            nc.sync.dma_start(w1p_t[:], w1p[:])

            x_tiles = []
            x8_tiles = {}

            def load_x(li):
                ncols = LOADS[li][1]
                x_t = xin.tile([KCH, KC16, ncols], mdt, tag=f"x{li}", name=f"x{li}")
                nc.sync.dma_start(x_t[:], xts[li][:])
                x_tiles.append(x_t)

            def load_x8(k):
                x8_t = xin.tile(
                    [KCH, KC8, X8COLS], f8, tag=f"x8_{k}", name=f"x8_{k}"
                )
                nc.sync.dma_start(x8_t[:], x8ts[k][:])
                x8_tiles[k] = x8_t

            # interleave the fp8 chunk loads so each block's fp8 data
            # arrives no later than its fp16 data
            load_x(0)
            load_x8(0)
            wrest_t = consts.tile([KCH, WRCOLS], mdt)
            nc.sync.dma_start(wrest_t[:], wrest[:])
            load_x(1)
            load_x(2)
            load_x8(1)
            load_x(3)
            load_x(4)
            load_x8(2)
            load_x(5)
            load_x(6)
            load_x8(3)
            for li in range(7, len(LOADS)):
                load_x(li)

            # bias vector on the (otherwise empty) scalar ring
            bd_t0 = consts.tile([HID, 2], f32)
            nc.scalar.dma_start(bd_t0[:], bd[:])

            w1_t = w1p_t.rearrange("k (c m) -> k c m", c=KC)
            wtl_t = wrest_t[:, TAILOFF : TAILOFF + HID]
            w2_t = wrest_t[:, W2OFF : W2OFF + OUT]
            xtl_t = wrest_t[:, XTLOFF:WRCOLS]
            b1_t = bd_t0[:, 0:1]
            b2_t = bd_t0[0:OUT, 1:2]

            # Pre-touch the bias tiles on their consumer engines (b1 on DVE,
            # b2 on ACT) so the relu / bias-add instructions don't need a
            # second sync-wait for the bias DMA (walrus: 1 wait per inst).
            b1_probe = consts.tile([1, 1], f32)
            nc.vector.tensor_copy(b1_probe[:], b1_t[0:1, 0:1])
            b2_probe = consts.tile([1, 1], f32)
            nc.scalar.copy(b2_probe[:], b2_t[0:1, 0:1])

            # Matmuls self-load their weights, so every semaphore wait lands
            # on the Matmult itself -- and walrus only allows one sync-wait
            # there. Tiny dummy bf16 ldweights "probes" reading 1 element of
            # a tile absorb the cross-engine waits into the PE's in-order
            # stream before each matmul group. The loaded garbage weight is
            # irrelevant (the real matmuls self-load).
            def probe(ap):
                if ap.dtype == f8:
                    nc.tensor.ldweights(ap[0:1, 0:1])
                else:
                    nc.tensor.ldweights(ap[0:1, 0:1].bitcast(mybir.dt.bfloat16))

            probe(w1_t[:, 0, :])
            probe(xtl_t[:])
            probe(w2_t[:])

            # HAM warm-up: dummy matmuls over a zeroed scratch tile bridge
            # the PE-idle window until the first x block lands, so the
            # clock gate is at 2.4 GHz for every real matmul. The dummy
            # PSUM bank is dedicated (not in the ps ring) so fillers never
            # serialize against real accumulations.
            scratch = consts.tile([HID, NB], mdt)
            nc.gpsimd.memset(scratch[:], 0.0)
            psd = psdp.tile([HID, NB], f32, tag="psd")

            def dummy_mms(n):
                for _ in range(n):
                    nc.tensor.matmul(
                        psd[:], scratch[:, 0:HID], scratch[:], start=True, stop=True
                    )

            dummy_mms(NWARM)

            # block bi (columns [bi*NB, bi*NB+NB)) -> (load idx, col offset)
            def block_view(bi):
                cs = bi * NB
                for li, (ls, ncols) in enumerate(LOADS):
                    if ls <= cs < ls + ncols:
                        return x_tiles[li], cs - ls
                raise AssertionError

            # Weight-stationary FC1 over groups of blocks: per group one
            # LDWEIGHTS per chunk serves every block in the group, so the
            # PE streams at its pure 216ns/matmul cadence with fewer
            # FC2/tail transition stalls. G=2 matches the 1024-col loads
            # and the 4-bank ps1 ring (group g+1 uses the other 2 banks
            # while group g's relus drain).
            def fc1_group(blocks):
                x_t, off0 = block_view(blocks[0])
                probe(x_t[:, 0, off0 : off0 + 1])
                x8_t = x8_tiles[(blocks[0] * NB) // X8COLS]
                probe(x8_t[:, 0, 0:1])
                pss = [
                    ps1p.tile([HID, NB], f32, tag="ps", name=f"ps{bi}")
                    for bi in blocks
                ]
                for c in range(KC16):
                    for ps_si, bi in zip(pss, blocks):
                        _, off = block_view(bi)
                        nc.tensor.matmul(
                            ps_si[:],
                            w1_t[:, c, :],
                            x_t[:, c, off : off + NB],
                            start=(c == 0),
                            stop=False,
                        )
                # chunks 4,5 stream as fp8 against the same fp16 weights
                # (mixed-dtype matmul, HW-verified exact)
                for c in range(KC16, KC):
                    for ps_si, bi in zip(pss, blocks):
                        o8 = (bi * NB) % X8COLS
                        nc.tensor.matmul(
                            ps_si[:],
                            w1_t[:, c, :],
                            x8_t[:, c - KC16, o8 : o8 + NB],
                            start=False,
                            stop=False,
                        )
                for ps_si, bi in zip(pss, blocks):
                    # 16-feature tail: K=16 matmul from a 32-aligned strip;
                    # a pair's two tails sit in different row groups so
                    # they execute concurrently (row tiling)
                    s = tail_strip(bi)
                    nc.tensor.matmul(
                        ps_si[:],
                        wtl_t[32 * s : 32 * s + KTAIL, :],
                        xtl_t[32 * s : 32 * s + KTAIL, ts(tail_colrange(bi), NB)],
                        start=False,
                        stop=True,
                        tile_position=(32 * s, 0),
                    )
                # relu right away: h = max(ps + b1, 0) -> SBUF fp16. The
                # pair's two relus alternate DVE / ACT so they run in
                # parallel and the FC2 pair waits half as long.
                hs = []
                for j, (ps_si, bi) in enumerate(zip(pss, blocks)):
                    h = hpool.tile([HID, NB], mdt, tag="h", name=f"h{bi}")
                    if j % 2 == 0:
                        nc.vector.tensor_scalar(
                            h[:],
                            ps_si[:],
                            b1_t[:],
                            0.0,
                            mybir.AluOpType.add,
                            mybir.AluOpType.max,
                        )
                    else:
                        nc.scalar.activation(
                            h[:],
                            ps_si[:],
                            mybir.ActivationFunctionType.Relu,
                            bias=b1_t[:],
                        )
                    hs.append(h)
                return hs

            def fc2_group(blocks, hs, last=False):
                for h in hs:
                    probe(h[:])
                # a pair's two FC2s pack into one PSUM bank at col strips
                # (0,0)/(0,32) and execute concurrently (col tiling)
                ps2 = ps2p.tile([HID, NB], f32, tag="ps2", name=f"o{blocks[0]}")
                for j, h in enumerate(hs):
                    nc.tensor.matmul(
                        ps2[32 * j : 32 * j + OUT, :],
                        w2_t[:],
                        h[:],
                        start=True,
                        stop=True,
                        tile_position=(0, 32 * j),
                    )
                o = opool.tile([HID, NB], f32, tag="o", name=f"yo{blocks[0]}")
                for j, bi in enumerate(blocks):
                    # FC2 bias stage also moves PSUM -> SBUF. Mid-kernel it
                    # runs on the (otherwise idle) scalar engine; the final
                    # group uses the DVE (80ns vs 677ns) to shorten the
                    # kernel's drain chain.
                    sl = slice(32 * j, 32 * j + OUT)
                    b2_j = bd_t0[sl, 1:2]  # b2 replicated per 32-strip
                    # bias + PSUM->SBUF move on the DVE: a [10, 512]
                    # tensor_scalar is ~80ns there vs ~677ns as an ACT
                    # activation
                    nc.vector.tensor_scalar(
                        o[sl, :],
                        ps2[sl, :],
                        b2_j,
                        0.0,
                        mybir.AluOpType.add,
                        mybir.AluOpType.bypass,
                    )
                    # store via SWDGE: the gpsimd engine is otherwise idle
                    # and its DMASW semaphore lanes are separate from the 8
                    # DMAHW lanes the x stream needs; issuing from ACT (v3)
                    # both serialized the tail behind per-store descriptor
                    # generation (measured 1.4us per HWDGE store issue on
                    # ACT vs 0.64us SWDGE) and burned DMAHW lanes.
                    nc.gpsimd.dma_start(yt[:, ts(bi, NB)], o[sl, :])

            # group blocks in pairs within each load (singleton for the
            # 512-col taper loads); software-pipeline FC2 one group behind
            groups = []
            for li, (cs, ncols) in enumerate(LOADS):
                b0 = cs // NB
                nb = ncols // NB
                for s in range(0, nb, 2):
                    groups.append([b0 + s + j for j in range(min(2, nb - s))])

            # While the DMA stream is still filling (first ~4 groups), the
            # PE idles ~1us at each group boundary waiting for the next x
            # load; HAM sees the low duty cycle and re-throttles to
            # 1.2 GHz for ~7us (measured in v9/v10 when it flapped).
            # Always-ready dummy matmuls after the early groups keep the
            # PE busy through those waits.
            FILLER = {0: 4, 1: 4, 2: 4, 3: 4, 4: 3, 5: 2}
            pending = None
            for gi, g in enumerate(groups):
                hs = fc1_group(g)
                if pending is not None:
                    fc2_group(*pending)
                pending = (g, hs)
                dummy_mms(FILLER.get(gi, 0))
            fc2_group(*pending, last=True)

    # This walrus build allows one sync-wait per instruction; Tile emits
    # multi-waits (e.g. slot-recycle WAW + readers-release on DMAs). Split
    # them into event-semaphore chains, same as bacc.compile() does.
    import bass_rust

    bass_rust.generate_event_semaphores(nc)
    return nc


def _fuse_conv_fc1(conv_w, w1):
    """W1e = w1 @ C where C is the 3x3 valid-conv operator [676, 784]."""
    cw = np.asarray(conv_w, np.float64).reshape(KH, KW)
    w1_r = np.asarray(w1, np.float64).reshape(HID, H - KH + 1, W - KW + 1)
    w1e = np.zeros((HID, H, W), np.float64)
    for a in range(KH):
        for b in range(KW):
            w1e[:, a : a + H - KH + 1, b : b + W - KW + 1] += w1_r * cw[a, b]
    return w1e.reshape(HID, CIN).astype(np.float32)


def _core_x(x_shard):
    """Pre-tile one core's x rows [BS, 784]: per-load fp16 tensors x{li}
    [128, 4, ncols] (features 0:512), whole-batch fp8 tensors x8_{k}
    [128, 2, 2048] (features 512:768), plus the packed fp16 tail xtl
    [128, 1024] (block bi's 16 tail features on partition strip
    16*(bi%8), columns 512*(bi//8))."""
    from ml_dtypes import float8_e4m3fn

    out = {}
    for li, (cs, ncols) in enumerate(LOADS):
        blk = x_shard[cs : cs + ncols, : KC16 * KCH].reshape(ncols, KC16, KCH)
        out[f"x{li}"] = np.ascontiguousarray(blk.transpose(2, 1, 0).astype(HOST_DT))
    x8 = x_shard[:, KC16 * KCH : KC * KCH].reshape(BS, KC8, KCH)
    for k in range(BS // X8COLS):
        out[f"x8_{k}"] = np.ascontiguousarray(
            x8[k * X8COLS : (k + 1) * X8COLS]
            .transpose(2, 1, 0)
            .astype(float8_e4m3fn)
        )
    xtl = np.zeros((KCH, (NBLK // NGRP) * NB), HOST_DT)
    tail = x_shard[:, KC * KCH :].astype(HOST_DT)  # [BS, 16]
    for bi in range(NBLK):
        g, cb = tail_strip(bi), tail_colrange(bi)
        xtl[32 * g : 32 * g + KTAIL, cb * NB : (cb + 1) * NB] = tail[
            bi * NB : (bi + 1) * NB
        ].T
    return out, xtl


def _host_weights(conv_w, w1, b1, w2, b2):
    """Pack fp16 weights into w1p [128, 768] + wrest [128, 2058] (xtl
    cols filled per-core) and biases into bd."""
    w1e_t = _fuse_conv_fc1(conv_w, w1).T.astype(HOST_DT)  # [784, 128]
    w2t = np.asarray(w2, np.float32).T.astype(HOST_DT)  # [128, 10]
    w1p = np.ascontiguousarray(
        w1e_t[: KC * KCH].reshape(KC, KCH, HID).transpose(1, 0, 2).reshape(KCH, -1)
    )
    wrest = np.zeros((KCH, WRCOLS), HOST_DT)
    # tail weight rows replicated on every 32-aligned strip: the tail
    # matmuls slice 16-row windows at their strip base
    tail_w = w1e_t[KC * KCH :]  # [16, 128]
    for g in range(NGRP):
        wrest[32 * g : 32 * g + KTAIL, TAILOFF : TAILOFF + HID] = tail_w
    wrest[:, W2OFF : W2OFF + OUT] = w2t
    bd = np.zeros((HID, 2), np.float32)
    bd[:, 0] = np.asarray(b1, np.float32)
    # b2 at both FC2 col-strip bases (partitions 0:10 and 32:42)
    bd[0:OUT, 1] = np.asarray(b2, np.float32)
    bd[32 : 32 + OUT, 1] = np.asarray(b2, np.float32)
    return w1p, wrest, np.ascontiguousarray(bd)


def _run(x, conv_w, w1, b1, w2, b2, trace=False):
    x = np.asarray(x, np.float32)
    w1p, wrest, bd = _host_weights(conv_w, w1, b1, w2, b2)

    nc = _build_nc()
    in_maps = []
    for c in range(NCORES):
        m, xtl = _core_x(x[c * BS : (c + 1) * BS])
        m["w1p"] = w1p
        m["wrest"] = np.ascontiguousarray(
            np.concatenate([wrest[:, :XTLOFF], xtl], axis=1)
        )
        m["bd"] = bd
        in_maps.append(m)
    res = run_bass_kernel_spmd(nc, in_maps, list(range(NCORES)), trace=trace)

    y = np.empty((B_TOTAL, OUT), np.float32)
    for c, r in enumerate(res.results):
        y[c * BS : (c + 1) * BS] = r["yt"].T
    return y, res


def kernel(x, conv_w, w1, b1, w2, b2):
    y, _ = _run(x, conv_w, w1, b1, w2, b2)
    return y


# revision 47
# speedup vs baseline: 1.0183x; 1.0183x over previous
"""Trainium2 Bass kernel for nn_DigitConvolutionalModel (dense_cnn).

Model: y = relu(conv3x3(x) @ w1.T + b1) @ w2.T + b2, x: [65536, 784] f32.

Strategy:
  * The 3x3 valid conv (784 -> 676) and FC1 (676 -> 128) are both linear,
    so they fuse on the host into one effective weight W1e = w1 @ C with
    shape [128, 784] (C is the sparse conv operator). The device then runs
    a pure GEMM pipeline: y = relu(x @ W1e.T + b1) @ w2.T + b2.
  * Pure data parallel over 8 NeuronCores: each core gets 8192 rows of x.
    No collectives; each core produces its own output shard.
  * Mixed-precision operands (all accumulation fp32 in PSUM): features
    0:512 travel fp16, features 512:768 travel fp8e4m3 against fp16
    weights (mixed-dtype matmul, HW-verified bit-exact vs the quantized
    reference), the 16-feature tail travels fp16. End-to-end rel_fro =
    1.402e-2 on the fixed inputs (deterministic; HW matches the numpy
    emulation to 4 digits), under the 2e-2 gate with 30% margin, and x
    HBM bytes drop from 12.6 to 10.6 MB/core. More fp8 (n8=400) would
    leave only 10% margin -- not taken.
  * Every x DMA uses exactly 128 partitions -> one contiguous
    descriptor per partition, port-matched by the HWDGE balancer to the
    16 SDMA engines (~25 GB/s/engine). Measured: any other partition
    count (16- or 112-partition tiles) breaks the port match and drops
    to ~16 GB/s/engine. fp16 loads taper 512,512,1024x6,512,512 cols
    (quick first block, short drain; 1024-col mid loads keep PE idle
    gaps under HAM's 3.4us re-throttle window -- 2048-col loads caused
    mid-kernel 1.2 GHz collapses). fp8 chunks ride 4 x [128, 2, 2048]
    loads interleaved into the stream. All loads are SBUF-resident (no
    recycling -> the stream never waits on compute). Sync-ring order
    [w1p, x0, x8_0, wrest, x1, ...] keeps total HWDGE DMAs at 16 so the
    8 round-robin DMAHW semaphore lanes only pair early-completing
    loads (lane reuse stalled loads 10-30us in earlier revisions).
  * The 16-feature tail for the whole batch travels inside wrest as a
    [128, 2048] block: block bi's tail sits on the 32-aligned partition
    strip 32*tail_strip(bi). A pair's two K=16 tail matmuls land in
    different row groups and execute concurrently via tile_position row
    tiling (~4ns apart, measured).
  * Blocks process in weight-stationary pairs (one 1024-col load): 12
    main matmuls stream at the PE's pure 216ns cadence, 2 concurrent
    tail matmuls, relu+bias (PSUM -> SBUF fp16) alternating DVE/ACT so
    the pair's two relus run in parallel, then the pair's two [10, 512]
    FC2 matmuls pack into one PSUM bank at col strips (0,0)/(0,32) and
    run concurrently. FC2 bias rides a DVE tensor_scalar (80ns vs 677ns
    on ACT). FC2 groups are emitted one group behind FC1 so relu
    latency hides under the next group's matmuls. Output returns as
    yT [10, 8192] per core; the host transposes.
  * Stores issue via SWDGE from the otherwise-idle gpsimd engine: its
    DMASW semaphore lanes are separate from the x stream's DMAHW lanes,
    and an ACT-issued HWDGE store costs 1.4us of sequencer time.
  * The PE HAM clock gate defaults to 1.2 GHz and needs ~3.4us of
    sustained activity to ramp to 2.4 GHz. Twelve dummy matmuls bridge
    the startup window (event-semaphore preamble + first loads), and
    always-ready filler matmuls after the first four groups keep the PE
    busy through early x-load waits -- without them HAM re-throttles
    mid-kernel for ~7us (measured).
  * Tile/walrus quirks handled explicitly: this walrus allows ONE sync
    wait per instruction, so multi-waits are split into event-semaphore
    chains (bass_rust.generate_event_semaphores) and tiny dummy
    ldweights "probes" absorb cross-engine waits into the PE stream
    ahead of each matmul group.
"""

import numpy as np

import concourse.bass as bass
import concourse.mybir as mybir
import concourse.tile as tile
from concourse.bass import ts
from concourse.bass_utils import run_bass_kernel_spmd

H = W = 28
KH = KW = 3
CIN = H * W  # 784
HID = 128
OUT = 10
B_TOTAL = 65536
NCORES = 8
BS = B_TOTAL // NCORES  # 8192 rows per core
NB = 512  # batch columns per psum block (fp32 PSUM bank limit)
NBLK = BS // NB  # 16
KCH = 128
KC = 6  # full chunks (6 * 128 = 768)
KC16 = 4  # chunks 0..3 (features 0:512) travel fp16
KC8 = KC - KC16  # chunks 4,5 (features 512:768) travel fp8e4m3
KTAIL = CIN - KC * KCH  # 16
NGRP = 4  # tail partition strips, at 32-aligned bases {0,32,64,96}
NWARM = 12  # HAM warm-up dummy matmuls
X8COLS = 2048  # columns per fp8 x load (4 loads cover the batch)


def tail_strip(bi):
    """Tail strip index for block bi: a pair [2k, 2k+1] lands on strips
    in different 32-row groups, so the two K=16 tail matmuls run
    concurrently via tile_position row tiling."""
    return 2 * (bi % 2) + (bi // 2) % 2


def tail_colrange(bi):
    return bi // NGRP

# x load schedule (column start, width). Fine-grained 1024-col loads in
# the middle: the PE consumes a 1024-col pair in ~3.9us while the DMA
# delivers one in ~3.7us, so the PE never idles past HAM's 3.4us MID
# window (a 2048-col load creates a >3.5us PE bubble and the clock gate
# drops to 1.2 GHz -- measured in v6). 512-col loads at both ends for a
# fast first block and a short drain. 12 HWDGE DMAs (w1p, x0, wrest,
# x1..x9) over 8 round-robin DMAHW lanes: the four lane reuses all pair
# a late x load with an early-completing load, so no issue stalls.
LOADS = (
    [(0, 512), (512, 512)]
    + [(cs, 1024) for cs in range(1024, BS - 1024, 1024)]
    + [(BS - 1024, 512), (BS - 512, 512)]
)
assert sum(n for _, n in LOADS) == BS

# weight DMAs split so the sync-ring order [w1p, x0, wrest, x1, ...]
# gates block 0's main FC1 only on the tiny w1p + x0 loads:
#   w1p  [128, 768]:  w1e main chunks [k, c, m] (c-major)
#   wrest [128, 2186]: cols 0:128 = tail weight rows replicated on all 4
#                      32-aligned strips (the tail matmuls slice 16-row
#                      windows, so one copy serves every strip),
#                      128:138 = w2t, 138:2186 = per-core packed x tail
W1COLS = 768
TAILOFF, W2OFF, XTLOFF, WRCOLS = 0, 128, 138, 2186

HOST_DT = np.float16


def _build_nc():
    f32 = mybir.dt.float32
    mdt = mybir.dt.float16
    f8 = mybir.dt.float8e4
    nc = bass.Bass()
    xts = [
        nc.dram_tensor(f"x{li}", [KCH, KC16, ncols], mdt, kind="ExternalInput")
        for li, (_, ncols) in enumerate(LOADS)
    ]
    x8ts = [
        nc.dram_tensor(f"x8_{k}", [KCH, KC8, X8COLS], f8, kind="ExternalInput")
        for k in range(BS // X8COLS)
    ]
    w1p = nc.dram_tensor("w1p", [KCH, W1COLS], mdt, kind="ExternalInput")
    wrest = nc.dram_tensor("wrest", [KCH, WRCOLS], mdt, kind="ExternalInput")
    # both biases in one f32 tensor: col 0 = b1, col 1 rows 0:10 = b2
    bd = nc.dram_tensor("bd", [HID, 2], f32, kind="ExternalInput")
    yt = nc.dram_tensor("yt", [OUT, BS], f32, kind="ExternalOutput")

    with tile.TileContext(nc) as tc:
        with (
            tc.tile_pool(name="consts", bufs=1) as consts,
            tc.tile_pool(name="xin", bufs=1) as xin,
            tc.tile_pool(name="hpool", bufs=8) as hpool,
            tc.tile_pool(name="opool", bufs=6) as opool,
            tc.tile_pool(name="ps1", bufs=4, space="PSUM") as ps1p,
            tc.tile_pool(name="ps2", bufs=2, space="PSUM") as ps2p,
            tc.tile_pool(name="psd", bufs=1, space="PSUM") as psdp,
        ):
            # Sync-ring order [w1p, x0, wrest, x1, ...]: the 0.23 MB main
            # weights land first (~10us), x0 right behind (~12us), and the
            # tail/w2/xtl pack arrives just before block 0's tail matmul
            # needs it. (v4's weights-on-scalar crawled behind the x
            # stream to ~18us; v5's full-pack-first pushed x0 late and let
            # HAM re-throttle during the 4us PE idle that followed.)
            w1p_t = consts.tile([KCH, W1COLS], mdt)
            nc.sync.dma_start(w1p_t[:], w1p[:])

            x_tiles = []
            x8_tiles = {}

            def load_x(li):
                ncols = LOADS[li][1]
                x_t = xin.tile([KCH, KC16, ncols], mdt, tag=f"x{li}", name=f"x{li}")
                nc.sync.dma_start(x_t[:], xts[li][:])
                x_tiles.append(x_t)

            def load_x8(k):
                x8_t = xin.tile(
                    [KCH, KC8, X8COLS], f8, tag=f"x8_{k}", name=f"x8_{k}"
                )
                nc.sync.dma_start(x8_t[:], x8ts[k][:])
                x8_tiles[k] = x8_t

            # interleave the fp8 chunk loads so each block's fp8 data
            # arrives no later than its fp16 data
            load_x(0)
            load_x8(0)
            wrest_t = consts.tile([KCH, WRCOLS], mdt)
            nc.sync.dma_start(wrest_t[:], wrest[:])
            load_x(1)
            load_x(2)
            load_x8(1)
            load_x(3)
            load_x(4)
            load_x8(2)
            load_x(5)
            load_x(6)
            load_x8(3)
            for li in range(7, len(LOADS)):
                load_x(li)

            # bias vector on the (otherwise empty) scalar ring
            bd_t0 = consts.tile([HID, 2], f32)
            nc.scalar.dma_start(bd_t0[:], bd[:])

            w1_t = w1p_t.rearrange("k (c m) -> k c m", c=KC)
            wtl_t = wrest_t[:, TAILOFF : TAILOFF + HID]
            w2_t = wrest_t[:, W2OFF : W2OFF + OUT]
            xtl_t = wrest_t[:, XTLOFF:WRCOLS]
            b1_t = bd_t0[:, 0:1]
            b2_t = bd_t0[0:OUT, 1:2]

            # Pre-touch the bias tiles on their consumer engines (b1 on DVE,
            # b2 on ACT) so the relu / bias-add instructions don't need a
            # second sync-wait for the bias DMA (walrus: 1 wait per inst).
            b1_probe = consts.tile([1, 1], f32)
            nc.vector.tensor_copy(b1_probe[:], b1_t[0:1, 0:1])
            b2_probe = consts.tile([1, 1], f32)
            nc.scalar.copy(b2_probe[:], b2_t[0:1, 0:1])

            # Matmuls self-load their weights, so every semaphore wait lands
            # on the Matmult itself -- and walrus only allows one sync-wait
            # there. Tiny dummy bf16 ldweights "probes" reading 1 element of
            # a tile absorb the cross-engine waits into the PE's in-order
            # stream before each matmul group. The loaded garbage weight is
            # irrelevant (the real matmuls self-load).
            def probe(ap):
                if ap.dtype == f8:
                    nc.tensor.ldweights(ap[0:1, 0:1])
                else:
                    nc.tensor.ldweights(ap[0:1, 0:1].bitcast(mybir.dt.bfloat16))

            probe(w1_t[:, 0, :])
            probe(xtl_t[:])
            probe(w2_t[:])

            # HAM warm-up: dummy matmuls over a zeroed scratch tile bridge
            # the PE-idle window until the first x block lands, so the
            # clock gate is at 2.4 GHz for every real matmul. The dummy
            # PSUM bank is dedicated (not in the ps ring) so fillers never
            # serialize against real accumulations.
            scratch = consts.tile([HID, NB], mdt)
            nc.gpsimd.memset(scratch[:], 0.0)
            psd = psdp.tile([HID, NB], f32, tag="psd")

            def dummy_mms(n):
                for _ in range(n):
                    nc.tensor.matmul(
                        psd[:], scratch[:, 0:HID], scratch[:], start=True, stop=True
                    )

            dummy_mms(NWARM)

            # block bi (columns [bi*NB, bi*NB+NB)) -> (load idx, col offset)
            def block_view(bi):
                cs = bi * NB
                for li, (ls, ncols) in enumerate(LOADS):
                    if ls <= cs < ls + ncols:
                        return x_tiles[li], cs - ls
                raise AssertionError

            # Weight-stationary FC1 over groups of blocks: per group one
            # LDWEIGHTS per chunk serves every block in the group, so the
            # PE streams at its pure 216ns/matmul cadence with fewer
            # FC2/tail transition stalls. G=2 matches the 1024-col loads
            # and the 4-bank ps1 ring (group g+1 uses the other 2 banks
            # while group g's relus drain).
            def fc1_group(blocks):
                x_t, off0 = block_view(blocks[0])
                probe(x_t[:, 0, off0 : off0 + 1])
                x8_t = x8_tiles[(blocks[0] * NB) // X8COLS]
                probe(x8_t[:, 0, 0:1])
                pss = [
                    ps1p.tile([HID, NB], f32, tag="ps", name=f"ps{bi}")
                    for bi in blocks
                ]
                for c in range(KC16):
                    for ps_si, bi in zip(pss, blocks):
                        _, off = block_view(bi)
                        nc.tensor.matmul(
                            ps_si[:],
                            w1_t[:, c, :],
                            x_t[:, c, off : off + NB],
                            start=(c == 0),
                            stop=False,
                        )
                # chunks 4,5 stream as fp8 against the same fp16 weights
                # (mixed-dtype matmul, HW-verified exact)
                for c in range(KC16, KC):
                    for ps_si, bi in zip(pss, blocks):
                        o8 = (bi * NB) % X8COLS
                        nc.tensor.matmul(
                            ps_si[:],
                            w1_t[:, c, :],
                            x8_t[:, c - KC16, o8 : o8 + NB],
                            start=False,
                            stop=False,
                        )
                for ps_si, bi in zip(pss, blocks):
                    # 16-feature tail: K=16 matmul from a 32-aligned strip;
                    # a pair's two tails sit in different row groups so
                    # they execute concurrently (row tiling)
                    s = tail_strip(bi)
                    nc.tensor.matmul(
                        ps_si[:],
                        wtl_t[32 * s : 32 * s + KTAIL, :],
                        xtl_t[32 * s : 32 * s + KTAIL, ts(tail_colrange(bi), NB)],
                        start=False,
                        stop=True,
                        tile_position=(32 * s, 0),
                    )
                # relu right away: h = max(ps + b1, 0) -> SBUF fp16. The
                # pair's two relus alternate DVE / ACT so they run in
                # parallel and the FC2 pair waits half as long.
                hs = []
                for j, (ps_si, bi) in enumerate(zip(pss, blocks)):
                    h = hpool.tile([HID, NB], mdt, tag="h", name=f"h{bi}")
                    if j % 2 == 0:
                        nc.vector.tensor_scalar(
                            h[:],
                            ps_si[:],
                            b1_t[:],
                            0.0,
                            mybir.AluOpType.add,
                            mybir.AluOpType.max,
                        )
                    else:
                        nc.scalar.activation(
                            h[:],
                            ps_si[:],
                            mybir.ActivationFunctionType.Relu,
                            bias=b1_t[:],
                        )
                    hs.append(h)
                return hs

            def fc2_group(blocks, hs, last=False):
                for h in hs:
                    probe(h[:])
                # a pair's two FC2s pack into one PSUM bank at col strips
                # (0,0)/(0,32) and execute concurrently (col tiling)
                ps2 = ps2p.tile([HID, NB], f32, tag="ps2", name=f"o{blocks[0]}")
                for j, h in enumerate(hs):
                    nc.tensor.matmul(
                        ps2[32 * j : 32 * j + OUT, :],
                        w2_t[:],
                        h[:],
                        start=True,
                        stop=True,
                        tile_position=(0, 32 * j),
                    )
                o = opool.tile([HID, NB], f32, tag="o", name=f"yo{blocks[0]}")
                for j, bi in enumerate(blocks):
                    # FC2 bias stage also moves PSUM -> SBUF. Mid-kernel it
                    # runs on the (otherwise idle) scalar engine; the final
                    # group uses the DVE (80ns vs 677ns) to shorten the
                    # kernel's drain chain.
                    sl = slice(32 * j, 32 * j + OUT)
                    b2_j = bd_t0[sl, 1:2]  # b2 replicated per 32-strip
                    # bias + PSUM->SBUF move on the DVE: a [10, 512]
                    # tensor_scalar is ~80ns there vs ~677ns as an ACT
                    # activation
                    nc.vector.tensor_scalar(
                        o[sl, :],
                        ps2[sl, :],
                        b2_j,
                        0.0,
                        mybir.AluOpType.add,
                        mybir.AluOpType.bypass,
                    )
                    # store via SWDGE: the gpsimd engine is otherwise idle
                    # and its DMASW semaphore lanes are separate from the 8
                    # DMAHW lanes the x stream needs; issuing from ACT (v3)
                    # both serialized the tail behind per-store descriptor
                    # generation (measured 1.4us per HWDGE store issue on
                    # ACT vs 0.64us SWDGE) and burned DMAHW lanes.
                    nc.gpsimd.dma_start(yt[:, ts(bi, NB)], o[sl, :])

            # group blocks in pairs within each load (singleton for the
            # 512-col taper loads); software-pipeline FC2 one group behind
            groups = []
            for li, (cs, ncols) in enumerate(LOADS):
                b0 = cs // NB
                nb = ncols // NB
                for s in range(0, nb, 2):
                    groups.append([b0 + s + j for j in range(min(2, nb - s))])

            # While the DMA stream is still filling (first ~4 groups), the
            # PE idles ~1us at each group boundary waiting for the next x
            # load; HAM sees the low duty cycle and re-throttles to
            # 1.2 GHz for ~7us (measured in v9/v10 when it flapped).
            # Always-ready dummy matmuls after the early groups keep the
            # PE busy through those waits.
            # {0:4,1:4,2:4,3:3} measured 55.2-55.9us over four runs;
            # extending fillers to groups 4-5 measured 56.5us (they
            # outlive the x-wait windows and push real matmuls).
            FILLER = {0: 4, 1: 4, 2: 4, 3: 3}
            pending = None
            for gi, g in enumerate(groups):
                hs = fc1_group(g)
                if pending is not None:
                    fc2_group(*pending)
                pending = (g, hs)
                dummy_mms(FILLER.get(gi, 0))
            fc2_group(*pending, last=True)

    # This walrus build allows one sync-wait per instruction; Tile emits
    # multi-waits (e.g. slot-recycle WAW + readers-release on DMAs). Split
    # them into event-semaphore chains, same as bacc.compile() does.
    import bass_rust

    bass_rust.generate_event_semaphores(nc)
    return nc


def _fuse_conv_fc1(conv_w, w1):
    """W1e = w1 @ C where C is the 3x3 valid-conv operator [676, 784]."""
    cw = np.asarray(conv_w, np.float64).reshape(KH, KW)
    w1_r = np.asarray(w1, np.float64).reshape(HID, H - KH + 1, W - KW + 1)
    w1e = np.zeros((HID, H, W), np.float64)
    for a in range(KH):
        for b in range(KW):
            w1e[:, a : a + H - KH + 1, b : b + W - KW + 1] += w1_r * cw[a, b]
    return w1e.reshape(HID, CIN).astype(np.float32)


def _core_x(x_shard):
    """Pre-tile one core's x rows [BS, 784]: per-load fp16 tensors x{li}
    [128, 4, ncols] (features 0:512), whole-batch fp8 tensors x8_{k}
    [128, 2, 2048] (features 512:768), plus the packed fp16 tail xtl
    [128, 1024] (block bi's 16 tail features on partition strip
    16*(bi%8), columns 512*(bi//8))."""
    from ml_dtypes import float8_e4m3fn

    out = {}
    for li, (cs, ncols) in enumerate(LOADS):
        blk = x_shard[cs : cs + ncols, : KC16 * KCH].reshape(ncols, KC16, KCH)
        out[f"x{li}"] = np.ascontiguousarray(blk.transpose(2, 1, 0).astype(HOST_DT))
    x8 = x_shard[:, KC16 * KCH : KC * KCH].reshape(BS, KC8, KCH)
    for k in range(BS // X8COLS):
        out[f"x8_{k}"] = np.ascontiguousarray(
            x8[k * X8COLS : (k + 1) * X8COLS]
            .transpose(2, 1, 0)
            .astype(float8_e4m3fn)
        )
    xtl = np.zeros((KCH, (NBLK // NGRP) * NB), HOST_DT)
    tail = x_shard[:, KC * KCH :].astype(HOST_DT)  # [BS, 16]
    for bi in range(NBLK):
        g, cb = tail_strip(bi), tail_colrange(bi)
        xtl[32 * g : 32 * g + KTAIL, cb * NB : (cb + 1) * NB] = tail[
            bi * NB : (bi + 1) * NB
        ].T
    return out, xtl


def _host_weights(conv_w, w1, b1, w2, b2):
    """Pack fp16 weights into w1p [128, 768] + wrest [128, 2058] (xtl
    cols filled per-core) and biases into bd."""
    w1e_t = _fuse_conv_fc1(conv_w, w1).T.astype(HOST_DT)  # [784, 128]
    w2t = np.asarray(w2, np.float32).T.astype(HOST_DT)  # [128, 10]
    w1p = np.ascontiguousarray(
        w1e_t[: KC * KCH].reshape(KC, KCH, HID).transpose(1, 0, 2).reshape(KCH, -1)
    )
    wrest = np.zeros((KCH, WRCOLS), HOST_DT)
    # tail weight rows replicated on every 32-aligned strip: the tail
    # matmuls slice 16-row windows at their strip base
    tail_w = w1e_t[KC * KCH :]  # [16, 128]
    for g in range(NGRP):
        wrest[32 * g : 32 * g + KTAIL, TAILOFF : TAILOFF + HID] = tail_w
    wrest[:, W2OFF : W2OFF + OUT] = w2t
    bd = np.zeros((HID, 2), np.float32)
    bd[:, 0] = np.asarray(b1, np.float32)
    # b2 at both FC2 col-strip bases (partitions 0:10 and 32:42)
    bd[0:OUT, 1] = np.asarray(b2, np.float32)
    bd[32 : 32 + OUT, 1] = np.asarray(b2, np.float32)
    return w1p, wrest, np.ascontiguousarray(bd)


def _run(x, conv_w, w1, b1, w2, b2, trace=False):
    x = np.asarray(x, np.float32)
    w1p, wrest, bd = _host_weights(conv_w, w1, b1, w2, b2)

    nc = _build_nc()
    in_maps = []
    for c in range(NCORES):
        m, xtl = _core_x(x[c * BS : (c + 1) * BS])
        m["w1p"] = w1p
        m["wrest"] = np.ascontiguousarray(
            np.concatenate([wrest[:, :XTLOFF], xtl], axis=1)
        )
        m["bd"] = bd
        in_maps.append(m)
    res = run_bass_kernel_spmd(nc, in_maps, list(range(NCORES)), trace=trace)

    y = np.empty((B_TOTAL, OUT), np.float32)
    for c, r in enumerate(res.results):
        y[c * BS : (c + 1) * BS] = r["yt"].T
    return y, res


def kernel(x, conv_w, w1, b1, w2, b2):
    y, _ = _run(x, conv_w, w1, b1, w2, b2)
    return y
